# revision 30
# baseline (speedup 1.0000x reference)
"""Trainium2 Bass kernel for nn_DCTTransform.

Computes, for every 32x32 patch P of x [B=32, NP=256, C=3, 32, 32]:
  dct_coeffs = D @ P @ D.T          (2D DCT, fp32)
  patch_grades[b,n] = sum_c sum_{ij} log1p(|dct|) * W[i,j]
where W = sum_k 2^k * bandpass_k.

Strategy (per core; x sharded 4 batches/core across 8 cores):
  - Contiguous 1MB DMA loads of 256 patches as [128, 2048] tiles
    (partition p holds patches 2p, 2p+1 of the megatile).
  - DVE stream-transpose (32x32 blocks) to put patch w-columns on
    partitions, then two PE matmuls against a block-diagonal D.T
    (4x 32x32 blocks) with a DVE block-transpose between them:
        U = D @ P^T   (via blockdiag matmul over swizzled AP)
        Y = D @ U^T = D P D^T
  - Two more DVE stream-transposes re-pack Y into the contiguous
    layout for a full-bandwidth 1MB store.
  - Grade path: abs (gpsimd) -> ln(1+.) (ACT) -> fused multiply+reduce
    against replicated W (tensor_tensor_reduce) giving one scalar per
    patch directly (patches are whole within a partition).
Host: assembles shards, sums the 3 channel grades.
"""

import sys

import numpy as np

PATCH = 32
B, NP, C = 32, 256, 3
N_CORES = 8
B_SHARD = B // N_CORES                      # 4 batches per core
PATCHES_PER_CORE = B_SHARD * NP * C         # 3072
MEGA_PATCHES = 256                          # patches per megatile
F = MEGA_PATCHES * PATCH * PATCH // 128     # 2048 free dim
N_MEGA = PATCHES_PER_CORE // MEGA_PATCHES   # 12


def _ensure_path():
    for p in ("/opt/trn_rl_repo", "/root/.axon_site/_ro/trn_rl_repo"):
        if p not in sys.path:
            sys.path.append(p)


def build_nc(n_mega=N_MEGA):
    """Build the single-core Bass program (SPMD across 8 cores)."""
    _ensure_path()
    import concourse.bacc as bacc
    import concourse.mybir as mybir
    from concourse.tile import TileContext

    f32 = mybir.dt.float32
    nc = bacc.Bacc("TRN2", target_bir_lowering=False)

    x = nc.dram_tensor("x", [n_mega, 128, F], f32, kind="ExternalInput")
    bd = nc.dram_tensor("bd", [128, 128], f32, kind="ExternalInput")
    wrep = nc.dram_tensor("wrep", [128, 1024], f32, kind="ExternalInput")
    y = nc.dram_tensor("y", [n_mega, 128, F], f32, kind="ExternalOutput")
    grades = nc.dram_tensor("grades", [128, 2 * n_mega], f32, kind="ExternalOutput")

    with TileContext(nc) as tc:
        with (
            tc.tile_pool(name="const", bufs=1) as cpool,
            tc.tile_pool(name="io", bufs=3) as iopool,
            tc.tile_pool(name="mid", bufs=3) as midpool,
            tc.tile_pool(name="gr", bufs=1) as grpool,
            tc.tile_pool(name="pp1", bufs=2, space="PSUM") as pp1,
            tc.tile_pool(name="pp2", bufs=2, space="PSUM") as pp2,
        ):
            bd_t = cpool.tile([128, 128], f32)
            nc.sync.dma_start(bd_t[:, :], bd[:, :])
            wrep_t = cpool.tile([128, 1024], f32)
            nc.sync.dma_start(wrep_t[:, :], wrep[:, :])
            grade_t = grpool.tile([128, 2 * n_mega], f32)

            for m in range(n_mega):
                # 1MB contiguous load: partition p = patches 2p, 2p+1
                tile_c = iopool.tile([128, F], f32, tag="tile_c")
                nc.sync.dma_start(tile_c[:, :], x[m])

                # T1: per-32x32-block transpose.
                # tt[32a+w, 32(32d+h)+c] = patch_{64a+2c+d}[h, w]
                tt = midpool.tile([128, F], f32, tag="tt")
                nc.vector.transpose(tt[:, :], tile_c[:, :])

                # view with free dims (c2, d, h) for the stage-1 matmul
                tt_v = tt.rearrange("p (d h c) -> p c d h", d=2, h=32, c=32)

                z = midpool.tile([128, F], f32, tag="z")
                for ch in range(4):
                    # MM1: U = D @ P^T per patch.
                    # ps1[32k+i, 32g+h] = U_q[i,h], q = 64k+16ch+g
                    ps1 = pp1.tile([128, 512], f32, tag="ps1")
                    nc.tensor.matmul(
                        ps1[:, :], bd_t[:, :], tt_v[:, 8 * ch : 8 * ch + 8, :, :]
                    )
                    # T2: ut[32k+h, 32g+i] = U^T[h, i]
                    ut = midpool.tile([128, 512], f32, tag="ut")
                    nc.vector.transpose(ut[:, :], ps1[:, :])
                    # MM2: Y = D @ U^T = D P D^T
                    # ps2[32k+i', 32g+j'] = Y_q[i', j']
                    ps2 = pp2.tile([128, 512], f32, tag="ps2")
                    nc.tensor.matmul(ps2[:, :], bd_t[:, :], ut[:, :])
                    # T3: z[32k+j', 512ch+32g+i'] = Y_q[i', j']
                    nc.vector.transpose(z[:, 512 * ch : 512 * ch + 512], ps2[:, :])

                # permute z cols (c dd i) -> (dd i c) so a plain block
                # transpose lands the contiguous patch layout.
                # z col = 64c + 32dd + i  (c=patch-ish idx, dd=half, i=row)
                z2 = midpool.tile([128, F], f32, tag="z2")
                nc.scalar.copy(
                    z2[:, :],
                    z.rearrange("p (c dd i) -> p dd i c", c=32, dd=2, i=32),
                )
                # T4: yc[p, 1024d+32i+j] = Y_{2p+d}[i, j]
                yc = iopool.tile([128, F], f32, tag="yc")
                nc.vector.transpose(yc[:, :], z2[:, :])
                nc.sync.dma_start(y[m], yc[:, :])

                # grade path: one scalar per patch (2 patches per partition)
                ab = midpool.tile([128, F], f32, tag="ab")
                nc.gpsimd.tensor_scalar(
                    ab[:, :], yc[:, :], 0.0, None, mybir.AluOpType.abs_max
                )
                la = midpool.tile([128, F], f32, tag="la")
                nc.scalar.activation(
                    la[:, :], ab[:, :], mybir.ActivationFunctionType.Ln, bias=1.0
                )
                lw = midpool.tile([128, 1024], f32, tag="lw")
                rsink = midpool.tile([128, 1024], f32, tag="rsink")
                for d in range(2):
                    nc.gpsimd.tensor_tensor(
                        lw[:, :],
                        la[:, 1024 * d : 1024 * d + 1024],
                        wrep_t[:, :],
                        mybir.AluOpType.mult,
                    )
                    nc.scalar.activation(
                        rsink[:, :],
                        lw[:, :],
                        mybir.ActivationFunctionType.Identity,
                        accum_out=grade_t[:, 2 * m + d : 2 * m + d + 1],
                    )

            nc.sync.dma_start(grades[:, :], grade_t[:, :])
    nc.finalize()
    return nc


def build_nc_v2(n_mega=N_MEGA):
    """v2: store dct in block layout (host de-transposes); grades via PE
    indicator-matmuls accumulated across megatiles in 2 PSUM banks.

    Per chunk ch (16 patches), block (k,g) at [32k+i', 32g+j']:
      psum2[32k+i', 32g+j'] = Y_q[i',j'],  q = 64k+16ch+g (within megatile)
    Store y[m, 32k+i', 512ch+32g+j'] = Y. Grade: abs->ln->*Wblk->PE row-sum
    into psum_gb[bank m//8][32ch + 4(m%8) + k, 32g+j'], reduced at the end.
    """
    _ensure_path()
    import concourse.bacc as bacc
    import concourse.mybir as mybir
    from concourse.tile import TileContext

    f32 = mybir.dt.float32
    nc = bacc.Bacc("TRN2", target_bir_lowering=False)

    n_banks = (n_mega + 7) // 8
    x = nc.dram_tensor("x", [n_mega, 128, F], f32, kind="ExternalInput")
    bd = nc.dram_tensor("bd", [128, 128], f32, kind="ExternalInput")
    wblk = nc.dram_tensor("wblk", [128, 2048], f32, kind="ExternalInput")
    ind = nc.dram_tensor("ind", [128, 8 * 32], f32, kind="ExternalInput")
    y = nc.dram_tensor("y", [n_mega, 128, F], f32, kind="ExternalOutput")
    grades = nc.dram_tensor(
        "grades", [128, 16 * n_banks], f32, kind="ExternalOutput"
    )

    with TileContext(nc) as tc:
        with (
            tc.tile_pool(name="const", bufs=1) as cpool,
            tc.tile_pool(name="io", bufs=3) as iopool,
            tc.tile_pool(name="mid", bufs=3) as midpool,
            tc.tile_pool(name="gr", bufs=1) as grpool,
            tc.tile_pool(name="pp1", bufs=2, space="PSUM") as pp1,
            tc.tile_pool(name="pp2", bufs=2, space="PSUM") as pp2,
            tc.tile_pool(name="ppg", bufs=1, space="PSUM") as ppg,
        ):
            bd_t = cpool.tile([128, 128], f32)
            nc.sync.dma_start(bd_t[:, :], bd[:, :])
            wblk_t = cpool.tile([128, 2048], f32)
            nc.sync.dma_start(wblk_t[:, :], wblk[:, :])
            ind_t = cpool.tile([128, 8 * 32], f32)
            nc.sync.dma_start(ind_t[:, :], ind[:, :])

            gb = [
                ppg.tile([128, 512], f32, tag=f"gb{b}", name=f"gb{b}")
                for b in range(n_banks)
            ]

            for m in range(n_mega):
                mp = m % 8
                gbank = gb[m // 8]
                tile_c = iopool.tile([128, F], f32, tag="tile_c")
                nc.sync.dma_start(tile_c[:, :], x[m])

                tt = midpool.tile([128, F], f32, tag="tt")
                nc.vector.transpose(tt[:, :], tile_c[:, :])
                tt_v = tt.rearrange("p (d h c) -> p c d h", d=2, h=32, c=32)

                sy = iopool.tile([128, F], f32, tag="sy")
                for ch in range(4):
                    ps1 = pp1.tile([128, 512], f32, tag="ps1")
                    nc.tensor.matmul(
                        ps1[:, :], bd_t[:, :], tt_v[:, 8 * ch : 8 * ch + 8, :, :]
                    )
                    ut = midpool.tile([128, 512], f32, tag="ut")
                    nc.vector.transpose(ut[:, :], ps1[:, :])
                    ps2 = pp2.tile([128, 512], f32, tag="ps2")
                    nc.tensor.matmul(ps2[:, :], bd_t[:, :], ut[:, :])
                    # signed copy for the store (ACT)
                    nc.scalar.copy(sy[:, 512 * ch : 512 * ch + 512], ps2[:, :])
                nc.sync.dma_start(y[m], sy[:, :])

                # grade path, batched per megatile:
                # abs (ACT) -> ln1p (ACT) -> *W (gpsimd) -> PE row-sums
                ab = midpool.tile([128, F], f32, tag="ab")
                nc.scalar.activation(
                    ab[:, :], sy[:, :], mybir.ActivationFunctionType.Abs
                )
                la = midpool.tile([128, F], f32, tag="la")
                nc.scalar.activation(
                    la[:, :], ab[:, :], mybir.ActivationFunctionType.Ln, bias=1.0
                )
                lw = midpool.tile([128, F], f32, tag="lw")
                nc.gpsimd.tensor_tensor(
                    lw[:, :], la[:, :], wblk_t[:, :], mybir.AluOpType.mult
                )
                for ch in range(4):
                    # row-sum over i' into the megatile's grade rows
                    nc.tensor.matmul(
                        gbank[32 * ch : 32 * ch + 32, :],
                        ind_t[:, 32 * mp : 32 * mp + 32],
                        lw[:, 512 * ch : 512 * ch + 512],
                        start=(mp == 0),
                        stop=(mp == 7 or m == n_mega - 1),
                        tile_position=(0, 32 * ch),
                        skip_group_check=True,
                    )

            # final j' reduction of the grade banks
            gr_s = grpool.tile([128, 16 * n_banks], f32)
            for b in range(n_banks):
                gcop = grpool.tile([128, 512], f32, tag="gcop")
                nc.scalar.copy(gcop[:, :], gb[b][:, :])
                nc.vector.tensor_reduce(
                    gr_s[:, 16 * b : 16 * b + 16],
                    gcop.rearrange("p (g j) -> p g j", g=16, j=32),
                    axis=mybir.AxisListType.X,
                    op=mybir.AluOpType.add,
                )
            nc.sync.dma_start(grades[:, :], gr_s[:, :])
    nc.finalize()
    return nc



def build_nc_v3(n_mega=N_MEGA):
    """v3: 16-way tile_position-packed fp32 matmuls (3.5x PE throughput).

    Stage 1 (2 rounds of 8 tiles): tile (r,s) computes, for the 16
    patches {64r+16s+t}, U^T... actually U = D @ P^T:
      bank_r[32s+i, 32t+h] = U_{64r+16s+t}[i, h]
    Stage 2 (16 tiles): tile at position (32ch, 32r) reads ut_r rows
    32ch..32ch+31 and writes
      psum2[32r+i', 512ch+32g+j'] = Y_{64r+16ch+g}[i', j']
    Grade: per-megatile ind-matmuls (M=4) -> gbm -> DVE j'-reduce.
    Store: block layout (host de-transposes).
    """
    _ensure_path()
    import concourse.bacc as bacc
    import concourse.mybir as mybir
    from concourse.tile import TileContext

    f32 = mybir.dt.float32
    nc = bacc.Bacc("TRN2", target_bir_lowering=False)

    x = nc.dram_tensor("x", [n_mega, 128, F], f32, kind="ExternalInput")
    dt_in = nc.dram_tensor("dt", [128, 34], f32, kind="ExternalInput")
    wblk = nc.dram_tensor("wblk", [128, 2048], f32, kind="ExternalInput")
    ind = nc.dram_tensor("ind", [128, 32], f32, kind="ExternalInput")
    y = nc.dram_tensor("y", [n_mega, 128, F], f32, kind="ExternalOutput")
    grades = nc.dram_tensor("grades", [128, 16 * n_mega], f32, kind="ExternalOutput")

    with TileContext(nc) as tc:
        with (
            tc.tile_pool(name="const", bufs=1) as cpool,
            tc.tile_pool(name="io", bufs=3) as iopool,
            tc.tile_pool(name="mid", bufs=3) as midpool,
            tc.tile_pool(name="gr", bufs=1) as grpool,
            tc.tile_pool(name="pp1", bufs=1, space="PSUM") as pp1,
            tc.tile_pool(name="pp2", bufs=1, space="PSUM") as pp2,
            tc.tile_pool(name="ppg", bufs=2, space="PSUM") as ppg,
        ):
            dt_t = cpool.tile([128, 34], f32)
            nc.sync.dma_start(dt_t[:, :], dt_in[:, :])
            wblk_t = cpool.tile([128, 2048], f32)
            nc.sync.dma_start(wblk_t[:, :], wblk[:, :])
            ind_t = cpool.tile([128, 32], f32)
            nc.sync.dma_start(ind_t[:, :], ind[:, :])
            gr_s = grpool.tile([128, 16 * n_mega], f32)

            pending = []

            def emit_grade(gm, glw):
                gbm = ppg.tile([128, 512], f32, tag="gbm", name=f"gbm{gm}")
                for ch in range(4):
                    nc.tensor.matmul(
                        gbm[32 * ch : 32 * ch + 32, :],
                        ind_t[:, :],
                        glw[:, 512 * ch : 512 * ch + 512],
                        tile_position=(0, 32 * ch),
                        skip_group_check=True,
                    )
                nc.vector.tensor_reduce(
                    gr_s[:, 16 * gm : 16 * gm + 16],
                    gbm.rearrange("p (g j) -> p g j", g=16, j=32),
                    axis=mybir.AxisListType.X,
                    op=mybir.AluOpType.add,
                )

            for m in range(n_mega):
                tile_c = iopool.tile([128, F], f32, tag="tile_c")
                nc.sync.dma_start(tile_c[:, :], x[m])

                tt = midpool.tile([128, F], f32, tag="tt")
                nc.vector.transpose(tt[:, :], tile_c[:, :])
                # tt[32a+w, 32(32d+h)+c] = patch_{64a+2c+d}[h, w]
                # tile (r,s) rhs col for (t=2u+d, h): 1024d + 32h + 8s + u
                tt_v = tt.rearrange("p (d h c) -> p c d h", d=2, h=32, c=32)

                uts = []
                for r in range(4):
                    bank = pp1.tile(
                        [128, 512], f32, tag=f"b{r % 2}", name=f"bank{r}_{m}"
                    )
                    for s in range(4):
                        nc.tensor.matmul(
                            bank[32 * s : 32 * s + 32, :],
                            dt_t[32 * r : 32 * r + 32, :32],
                            tt_v[32 * r : 32 * r + 32, 8 * s : 8 * s + 8, :, :],
                            tile_position=(32 * r, 32 * s),
                            skip_group_check=True,
                        )
                    ut = midpool.tile([128, 512], f32, tag=f"ut{r}", name=f"ut{r}_{m}")
                    nc.vector.transpose(ut[:, :], bank[:, :])
                    uts.append(ut)

                ps2 = pp2.tile([128, F], f32, tag="ps2")
                for ch in range(4):
                    for r in range(4):
                        nc.tensor.matmul(
                            ps2[32 * r : 32 * r + 32, 512 * ch : 512 * ch + 512],
                            dt_t[32 * ch : 32 * ch + 32, :32],
                            uts[r][32 * ch : 32 * ch + 32, :],
                            tile_position=(32 * ch, 32 * r),
                            skip_group_check=True,
                        )

                sy = iopool.tile([128, F], f32, tag="sy")
                nc.scalar.copy(sy[:, :], ps2[:, :])
                nc.sync.dma_start(y[m], sy[:, :])

                ab = midpool.tile([128, F], f32, tag="ab")
                nc.scalar.activation(
                    ab[:, :], sy[:, :], mybir.ActivationFunctionType.Abs
                )
                la = midpool.tile([128, F], f32, tag="la")
                nc.scalar.activation(
                    la[:, :], ab[:, :], mybir.ActivationFunctionType.Ln, bias=1.0
                )
                lw = midpool.tile([128, F], f32, tag="lw", name=f"lw{m}")
                nc.gpsimd.tensor_tensor(
                    lw[:, :], la[:, :], wblk_t[:, :], mybir.AluOpType.mult
                )
                pending.append((m, lw))

                # emit the PREVIOUS megatile's grade matmuls here so the
                # in-order PE never waits on the abs->ln->mult chain
                if len(pending) > 1:
                    emit_grade(*pending.pop(0))

            while pending:
                emit_grade(*pending.pop(0))
            nc.sync.dma_start(grades[:, :], gr_s[:, :])
    nc.finalize()
    return nc



def build_nc_v4(n_mega=N_MEGA):
    """v4: software-pipelined v3.

    PE emission order per m: s1A(m) [12 tiles, r=0..2] -> s2(m-1) [16] ->
    s1B(m) [4 tiles, r=3] -> ind(m-2) [4], so the PE stream is continuous
    (keeps HAM un-throttled at 2.4 GHz) and never waits on the DVE
    transposes or the grade elementwise chain.
    PSUM: 3 stage-1 banks (r3 reuses b0) + 4 stage-2 + 1 grade = 8.
    """
    _ensure_path()
    import concourse.bacc as bacc
    import concourse.mybir as mybir
    from concourse.tile import TileContext

    f32 = mybir.dt.float32
    bf16 = mybir.dt.bfloat16
    nc = bacc.Bacc("TRN2", target_bir_lowering=False)

    x = nc.dram_tensor("x", [n_mega, 128, F], f32, kind="ExternalInput")
    dt_in = nc.dram_tensor("dt", [128, 34], f32, kind="ExternalInput")
    wblk = nc.dram_tensor("wblk", [128, 2048], f32, kind="ExternalInput")
    ind = nc.dram_tensor("ind", [128, 32], bf16, kind="ExternalInput")
    y = nc.dram_tensor("y", [n_mega, 128, F], f32, kind="ExternalOutput")
    grades = nc.dram_tensor("grades", [128, 16 * n_mega], f32, kind="ExternalOutput")

    with TileContext(nc) as tc:
        with (
            tc.tile_pool(name="const", bufs=1) as cpool,
            tc.tile_pool(name="io", bufs=3) as iopool,
            tc.tile_pool(name="mid", bufs=3) as midpool,
            tc.tile_pool(name="gr", bufs=1) as grpool,
            tc.tile_pool(name="pp1", bufs=1, space="PSUM") as pp1,
            tc.tile_pool(name="pp2", bufs=1, space="PSUM") as pp2,
            tc.tile_pool(name="ppg", bufs=1, space="PSUM") as ppg,
        ):
            dt_t = cpool.tile([128, 34], f32)
            wblk_t = cpool.tile([128, 2048], f32)
            ind_t = cpool.tile([128, 32], bf16)
            gr_s = grpool.tile([128, 16 * n_mega], f32)
            consts_loaded = []

            def emit_const_loads():
                nc.sync.dma_start(dt_t[:, :], dt_in[:, :])
                nc.sync.dma_start(ind_t[:, :], ind[:, :])
                nc.sync.dma_start(wblk_t[:, :], wblk[:, :])
                consts_loaded.append(True)

            BTAG = ["b0", "b1", "b2", "gbm"]

            def emit_load_t1(m, nchunk):
                tt = midpool.tile([128, F], f32, tag="tt", name=f"tt{m}")
                cw = F // nchunk
                for c in range(nchunk):
                    sl = slice(cw * c, cw * c + cw)
                    if nchunk > 1:
                        tc_c = iopool.tile(
                            [128, cw], f32, tag=f"tcc{c}", name=f"tcc{m}_{c}"
                        )
                        nc.sync.dma_start(tc_c[:, :], x[m, :, sl])
                        if not consts_loaded:
                            emit_const_loads()
                        nc.vector.transpose(tt[:, sl], tc_c[:, :])
                    else:
                        tile_c = iopool.tile(
                            [128, F], f32, tag="tile_c", name=f"tc{m}"
                        )
                        nc.sync.dma_start(tile_c[:, :], x[m])
                        if not consts_loaded:
                            emit_const_loads()
                        nc.vector.transpose(tt[:, :], tile_c[:, :])
                return tt.rearrange("p (d h c) -> p c d h", d=2, h=32, c=32)

            def emit_s1(m, tt_v, rs):
                outs = []
                for r in rs:
                    pool = ppg if r == 3 else pp1
                    bank = pool.tile(
                        [128, 512], f32, tag=BTAG[r], name=f"bank{r}_{m}"
                    )
                    for s in range(4):
                        nc.tensor.matmul(
                            bank[32 * s : 32 * s + 32, :],
                            dt_t[32 * r : 32 * r + 32, :32],
                            tt_v[32 * r : 32 * r + 32, 8 * s : 8 * s + 8, :, :],
                            tile_position=(32 * r, 32 * s),
                            skip_group_check=True,
                        )
                    ut = midpool.tile([128, 512], f32, tag=f"ut{r}", name=f"ut{r}_{m}")
                    nc.vector.transpose(ut[:, :], bank[:, :])
                    outs.append(ut)
                return outs

            def emit_s2(m, uts):
                ps2 = pp2.tile([128, F], f32, tag="ps2", name=f"ps2_{m}")
                for ch in range(4):
                    for r in range(4):
                        nc.tensor.matmul(
                            ps2[32 * r : 32 * r + 32, 512 * ch : 512 * ch + 512],
                            dt_t[32 * ch : 32 * ch + 32, :32],
                            uts[r][32 * ch : 32 * ch + 32, :],
                            tile_position=(32 * ch, 32 * r),
                            skip_group_check=True,
                        )
                return ps2

            def emit_tail(m, ps2, nchunk):
                sy = iopool.tile([128, F], f32, tag="sy", name=f"sy{m}")
                ab = midpool.tile([128, F], f32, tag="ab", name=f"ab{m}")
                la = midpool.tile([128, F], bf16, tag="la", name=f"la{m}")
                lw = midpool.tile([128, F], bf16, tag="lw", name=f"lw{m}", bufs=4)
                cw = F // nchunk
                for c in range(nchunk):
                    sl = slice(cw * c, cw * c + cw)
                    if nchunk == 1:
                        nc.scalar.copy(sy[:, :1536], ps2[:, :1536])
                        nc.vector.tensor_copy(sy[:, 1536:], ps2[:, 1536:])
                    else:
                        nc.scalar.copy(sy[:, sl], ps2[:, sl])
                    nc.sync.dma_start(y[m, :, sl], sy[:, sl])
                    nc.scalar.activation(
                        ab[:, sl],
                        sy[:, sl],
                        mybir.ActivationFunctionType.Abs,
                        bias=dt_t[:, 33:34],
                    )
                    nc.scalar.activation(
                        la[:, sl],
                        ab[:, sl],
                        mybir.ActivationFunctionType.Ln,
                        bias=dt_t[:, 32:33],
                    )
                    nc.gpsimd.tensor_tensor(
                        lw[:, sl], la[:, sl], wblk_t[:, sl], mybir.AluOpType.mult
                    )
                return lw

            def emit_grade(gm, glw):
                gbm = ppg.tile([128, 512], f32, tag="gbm", name=f"gbm{gm}")
                for ch in range(4):
                    nc.tensor.matmul(
                        gbm[32 * ch : 32 * ch + 32, :],
                        ind_t[:, :],
                        glw[:, 512 * ch : 512 * ch + 512],
                        tile_position=(0, 32 * ch),
                        skip_group_check=True,
                    )
                nc.vector.tensor_reduce(
                    gr_s[:, 16 * gm : 16 * gm + 16],
                    gbm.rearrange("p (g j) -> p g j", g=16, j=32),
                    axis=mybir.AxisListType.X,
                    op=mybir.AluOpType.add,
                )

            uts_prev = None
            lw_pend = []
            for m in range(n_mega):
                tt_v = emit_load_t1(m, 4 if m == 0 else 1)
                uts = emit_s1(m, tt_v, (0, 1, 2, 3))
                if uts_prev is not None:
                    ps2 = emit_s2(m - 1, uts_prev)
                    lw_pend.append((m - 1, emit_tail(m - 1, ps2, 1)))
                if len(lw_pend) > 2:
                    emit_grade(*lw_pend.pop(0))
                uts_prev = uts

            ps2 = emit_s2(n_mega - 1, uts_prev)
            lw_pend.append((n_mega - 1, emit_tail(n_mega - 1, ps2, 4)))
            while lw_pend:
                emit_grade(*lw_pend.pop(0))
            nc.sync.dma_start(grades[:, :], gr_s[:, :])
    nc.finalize()
    return nc


def make_consts_v3(dct_matrix, bandpass_filters):
    D = np.asarray(dct_matrix, np.float32)
    bf = np.asarray(bandpass_filters, np.float32)
    dt = np.tile(D.T, (4, 1)).astype(np.float32)           # [128, 32]
    dt = np.concatenate(
        [dt, np.ones((128, 1), np.float32), np.zeros((128, 1), np.float32)], axis=1
    )                                                      # [128, 34]
    kw = (2.0 ** np.arange(bf.shape[0])).astype(np.float32)
    W = np.einsum("k,khw->hw", kw, bf).astype(np.float32)
    wblk = np.tile(W, (4, 64)).astype(np.float32)          # [128, 2048]
    import ml_dtypes

    ind = np.zeros((128, 32), np.float32)
    for k in range(4):
        ind[32 * k : 32 * k + 32, k] = 1.0
    return dt, wblk, ind.astype(ml_dtypes.bfloat16)


def unscramble_grades_v3(gr, n_mega=N_MEGA):
    """[128, 16*n_mega] -> per-patch sums in t order.

    gr[32ch+k, 16m+g] = grade of patch q = 256m + 64k + 16ch + g
    (rows 32ch+4.. are garbage)."""
    a = gr.reshape(4, 32, n_mega, 16)[:, :4]   # (ch, k, m, g)
    a = a.transpose(2, 1, 0, 3).reshape(-1)    # (m, k, ch, g) -> t
    return a


def make_consts_v2(dct_matrix, bandpass_filters):
    D = np.asarray(dct_matrix, np.float32)
    bf = np.asarray(bandpass_filters, np.float32)
    BD = np.zeros((128, 128), np.float32)
    for k in range(4):
        BD[32 * k : 32 * k + 32, 32 * k : 32 * k + 32] = D.T
    kw = (2.0 ** np.arange(bf.shape[0])).astype(np.float32)
    W = np.einsum("k,khw->hw", kw, bf).astype(np.float32)
    wblk = np.tile(W, (4, 64)).astype(np.float32)          # [128, 2048]
    ind = np.zeros((128, 8, 32), np.float32)
    for k in range(4):
        for mp in range(8):
            ind[32 * k : 32 * k + 32, mp, 4 * mp + k] = 1.0
    return BD, wblk, np.ascontiguousarray(ind.reshape(128, 256))


def unscramble_y_v2(y_dev, n_mega=N_MEGA):
    """[n_mega,128,2048] block layout -> [npatch,32,32] in t order."""
    a = y_dev.reshape(n_mega, 4, 32, 4, 16, 32)  # (m, k, i, ch, g, j)
    a = a.transpose(0, 1, 3, 4, 2, 5)            # (m, k, ch, g, i, j)
    return np.ascontiguousarray(a).reshape(n_mega * 256, 32, 32)


def unscramble_grades_v2(gr, n_mega=N_MEGA):
    """[128, 16*n_banks] -> per-patch sums [n_mega*256] in t order."""
    n_banks = (n_mega + 7) // 8
    a = gr.reshape(4, 8, 4, n_banks, 16)         # (ch, mp, k, b, g)
    a = a.transpose(3, 1, 2, 0, 4).reshape(-1)   # t = 256*(8b+mp)+64k+16ch+g
    return a[: n_mega * 256]


def make_consts(dct_matrix, bandpass_filters):
    D = np.asarray(dct_matrix, np.float32)
    bf = np.asarray(bandpass_filters, np.float32)
    BD = np.zeros((128, 128), np.float32)
    for k in range(4):
        BD[32 * k : 32 * k + 32, 32 * k : 32 * k + 32] = D.T
    kw = (2.0 ** np.arange(bf.shape[0])).astype(np.float32)
    W = np.einsum("k,khw->hw", kw, bf).astype(np.float32)
    wrep = np.tile(W.reshape(1, 1024), (128, 1)).astype(np.float32)
    return BD, wrep


_NC_CACHE = {}


def _get_nc(n_mega=N_MEGA):
    if n_mega not in _NC_CACHE:
        _NC_CACHE[n_mega] = build_nc_v4(n_mega)
    return _NC_CACHE[n_mega]


def kernel(x, dct_matrix, bandpass_filters):
    _ensure_path()
    from concourse.bass_utils import run_bass_kernel_spmd

    x = np.asarray(x, np.float32)
    DT, wblk, ind = make_consts_v3(dct_matrix, bandpass_filters)
    nc = _get_nc()

    in_maps = []
    for c in range(N_CORES):
        shard = np.ascontiguousarray(
            x[c * B_SHARD : (c + 1) * B_SHARD]
        ).reshape(N_MEGA, 128, F)
        in_maps.append({"x": shard, "dt": DT, "wblk": wblk, "ind": ind})

    res = run_bass_kernel_spmd(nc, in_maps, core_ids=list(range(N_CORES)))

    dct_parts, grade_parts = [], []
    for c in range(N_CORES):
        yc = unscramble_y_v2(res.results[c]["y"]).reshape(
            B_SHARD, NP, C, PATCH, PATCH
        )
        dct_parts.append(yc)
        ps = unscramble_grades_v3(res.results[c]["grades"])  # t-order
        grade_parts.append(ps.reshape(B_SHARD, NP, C).sum(-1))
    dct_coeffs = np.concatenate(dct_parts, axis=0).astype(np.float32)
    patch_grades = np.concatenate(grade_parts, axis=0).astype(np.float32)
    return dct_coeffs, patch_grades


# revision 31
# speedup vs baseline: 1.0426x; 1.0426x over previous
"""Trainium2 Bass kernel for nn_DCTTransform.

Computes, for every 32x32 patch P of x [B=32, NP=256, C=3, 32, 32]:
  dct_coeffs = D @ P @ D.T          (2D DCT, fp32)
  patch_grades[b,n] = sum_c sum_{ij} log1p(|dct|) * W[i,j]
where W = sum_k 2^k * bandpass_k.

Strategy (per core; x sharded 4 batches/core across 8 cores):
  - Contiguous 1MB DMA loads of 256 patches as [128, 2048] tiles
    (partition p holds patches 2p, 2p+1 of the megatile).
  - DVE stream-transpose (32x32 blocks) to put patch w-columns on
    partitions, then two PE matmuls against a block-diagonal D.T
    (4x 32x32 blocks) with a DVE block-transpose between them:
        U = D @ P^T   (via blockdiag matmul over swizzled AP)
        Y = D @ U^T = D P D^T
  - Two more DVE stream-transposes re-pack Y into the contiguous
    layout for a full-bandwidth 1MB store.
  - Grade path: abs (gpsimd) -> ln(1+.) (ACT) -> fused multiply+reduce
    against replicated W (tensor_tensor_reduce) giving one scalar per
    patch directly (patches are whole within a partition).
Host: assembles shards, sums the 3 channel grades.
"""

import sys

import numpy as np

PATCH = 32
B, NP, C = 32, 256, 3
N_CORES = 8
B_SHARD = B // N_CORES                      # 4 batches per core
PATCHES_PER_CORE = B_SHARD * NP * C         # 3072
MEGA_PATCHES = 256                          # patches per megatile
F = MEGA_PATCHES * PATCH * PATCH // 128     # 2048 free dim
N_MEGA = PATCHES_PER_CORE // MEGA_PATCHES   # 12


def _ensure_path():
    for p in ("/opt/trn_rl_repo", "/root/.axon_site/_ro/trn_rl_repo"):
        if p not in sys.path:
            sys.path.append(p)


def build_nc(n_mega=N_MEGA):
    """Build the single-core Bass program (SPMD across 8 cores)."""
    _ensure_path()
    import concourse.bacc as bacc
    import concourse.mybir as mybir
    from concourse.tile import TileContext

    f32 = mybir.dt.float32
    nc = bacc.Bacc("TRN2", target_bir_lowering=False)

    x = nc.dram_tensor("x", [n_mega, 128, F], f32, kind="ExternalInput")
    bd = nc.dram_tensor("bd", [128, 128], f32, kind="ExternalInput")
    wrep = nc.dram_tensor("wrep", [128, 1024], f32, kind="ExternalInput")
    y = nc.dram_tensor("y", [n_mega, 128, F], f32, kind="ExternalOutput")
    grades = nc.dram_tensor("grades", [128, 2 * n_mega], f32, kind="ExternalOutput")

    with TileContext(nc) as tc:
        with (
            tc.tile_pool(name="const", bufs=1) as cpool,
            tc.tile_pool(name="io", bufs=3) as iopool,
            tc.tile_pool(name="mid", bufs=3) as midpool,
            tc.tile_pool(name="gr", bufs=1) as grpool,
            tc.tile_pool(name="pp1", bufs=2, space="PSUM") as pp1,
            tc.tile_pool(name="pp2", bufs=2, space="PSUM") as pp2,
        ):
            bd_t = cpool.tile([128, 128], f32)
            nc.sync.dma_start(bd_t[:, :], bd[:, :])
            wrep_t = cpool.tile([128, 1024], f32)
            nc.sync.dma_start(wrep_t[:, :], wrep[:, :])
            grade_t = grpool.tile([128, 2 * n_mega], f32)

            for m in range(n_mega):
                # 1MB contiguous load: partition p = patches 2p, 2p+1
                tile_c = iopool.tile([128, F], f32, tag="tile_c")
                nc.sync.dma_start(tile_c[:, :], x[m])

                # T1: per-32x32-block transpose.
                # tt[32a+w, 32(32d+h)+c] = patch_{64a+2c+d}[h, w]
                tt = midpool.tile([128, F], f32, tag="tt")
                nc.vector.transpose(tt[:, :], tile_c[:, :])

                # view with free dims (c2, d, h) for the stage-1 matmul
                tt_v = tt.rearrange("p (d h c) -> p c d h", d=2, h=32, c=32)

                z = midpool.tile([128, F], f32, tag="z")
                for ch in range(4):
                    # MM1: U = D @ P^T per patch.
                    # ps1[32k+i, 32g+h] = U_q[i,h], q = 64k+16ch+g
                    ps1 = pp1.tile([128, 512], f32, tag="ps1")
                    nc.tensor.matmul(
                        ps1[:, :], bd_t[:, :], tt_v[:, 8 * ch : 8 * ch + 8, :, :]
                    )
                    # T2: ut[32k+h, 32g+i] = U^T[h, i]
                    ut = midpool.tile([128, 512], f32, tag="ut")
                    nc.vector.transpose(ut[:, :], ps1[:, :])
                    # MM2: Y = D @ U^T = D P D^T
                    # ps2[32k+i', 32g+j'] = Y_q[i', j']
                    ps2 = pp2.tile([128, 512], f32, tag="ps2")
                    nc.tensor.matmul(ps2[:, :], bd_t[:, :], ut[:, :])
                    # T3: z[32k+j', 512ch+32g+i'] = Y_q[i', j']
                    nc.vector.transpose(z[:, 512 * ch : 512 * ch + 512], ps2[:, :])

                # permute z cols (c dd i) -> (dd i c) so a plain block
                # transpose lands the contiguous patch layout.
                # z col = 64c + 32dd + i  (c=patch-ish idx, dd=half, i=row)
                z2 = midpool.tile([128, F], f32, tag="z2")
                nc.scalar.copy(
                    z2[:, :],
                    z.rearrange("p (c dd i) -> p dd i c", c=32, dd=2, i=32),
                )
                # T4: yc[p, 1024d+32i+j] = Y_{2p+d}[i, j]
                yc = iopool.tile([128, F], f32, tag="yc")
                nc.vector.transpose(yc[:, :], z2[:, :])
                nc.sync.dma_start(y[m], yc[:, :])

                # grade path: one scalar per patch (2 patches per partition)
                ab = midpool.tile([128, F], f32, tag="ab")
                nc.gpsimd.tensor_scalar(
                    ab[:, :], yc[:, :], 0.0, None, mybir.AluOpType.abs_max
                )
                la = midpool.tile([128, F], f32, tag="la")
                nc.scalar.activation(
                    la[:, :], ab[:, :], mybir.ActivationFunctionType.Ln, bias=1.0
                )
                lw = midpool.tile([128, 1024], f32, tag="lw")
                rsink = midpool.tile([128, 1024], f32, tag="rsink")
                for d in range(2):
                    nc.gpsimd.tensor_tensor(
                        lw[:, :],
                        la[:, 1024 * d : 1024 * d + 1024],
                        wrep_t[:, :],
                        mybir.AluOpType.mult,
                    )
                    nc.scalar.activation(
                        rsink[:, :],
                        lw[:, :],
                        mybir.ActivationFunctionType.Identity,
                        accum_out=grade_t[:, 2 * m + d : 2 * m + d + 1],
                    )

            nc.sync.dma_start(grades[:, :], grade_t[:, :])
    nc.finalize()
    return nc


def build_nc_v2(n_mega=N_MEGA):
    """v2: store dct in block layout (host de-transposes); grades via PE
    indicator-matmuls accumulated across megatiles in 2 PSUM banks.

    Per chunk ch (16 patches), block (k,g) at [32k+i', 32g+j']:
      psum2[32k+i', 32g+j'] = Y_q[i',j'],  q = 64k+16ch+g (within megatile)
    Store y[m, 32k+i', 512ch+32g+j'] = Y. Grade: abs->ln->*Wblk->PE row-sum
    into psum_gb[bank m//8][32ch + 4(m%8) + k, 32g+j'], reduced at the end.
    """
    _ensure_path()
    import concourse.bacc as bacc
    import concourse.mybir as mybir
    from concourse.tile import TileContext

    f32 = mybir.dt.float32
    nc = bacc.Bacc("TRN2", target_bir_lowering=False)

    n_banks = (n_mega + 7) // 8
    x = nc.dram_tensor("x", [n_mega, 128, F], f32, kind="ExternalInput")
    bd = nc.dram_tensor("bd", [128, 128], f32, kind="ExternalInput")
    wblk = nc.dram_tensor("wblk", [128, 2048], f32, kind="ExternalInput")
    ind = nc.dram_tensor("ind", [128, 8 * 32], f32, kind="ExternalInput")
    y = nc.dram_tensor("y", [n_mega, 128, F], f32, kind="ExternalOutput")
    grades = nc.dram_tensor(
        "grades", [128, 16 * n_banks], f32, kind="ExternalOutput"
    )

    with TileContext(nc) as tc:
        with (
            tc.tile_pool(name="const", bufs=1) as cpool,
            tc.tile_pool(name="io", bufs=3) as iopool,
            tc.tile_pool(name="mid", bufs=3) as midpool,
            tc.tile_pool(name="gr", bufs=1) as grpool,
            tc.tile_pool(name="pp1", bufs=2, space="PSUM") as pp1,
            tc.tile_pool(name="pp2", bufs=2, space="PSUM") as pp2,
            tc.tile_pool(name="ppg", bufs=1, space="PSUM") as ppg,
        ):
            bd_t = cpool.tile([128, 128], f32)
            nc.sync.dma_start(bd_t[:, :], bd[:, :])
            wblk_t = cpool.tile([128, 2048], f32)
            nc.sync.dma_start(wblk_t[:, :], wblk[:, :])
            ind_t = cpool.tile([128, 8 * 32], f32)
            nc.sync.dma_start(ind_t[:, :], ind[:, :])

            gb = [
                ppg.tile([128, 512], f32, tag=f"gb{b}", name=f"gb{b}")
                for b in range(n_banks)
            ]

            for m in range(n_mega):
                mp = m % 8
                gbank = gb[m // 8]
                tile_c = iopool.tile([128, F], f32, tag="tile_c")
                nc.sync.dma_start(tile_c[:, :], x[m])

                tt = midpool.tile([128, F], f32, tag="tt")
                nc.vector.transpose(tt[:, :], tile_c[:, :])
                tt_v = tt.rearrange("p (d h c) -> p c d h", d=2, h=32, c=32)

                sy = iopool.tile([128, F], f32, tag="sy")
                for ch in range(4):
                    ps1 = pp1.tile([128, 512], f32, tag="ps1")
                    nc.tensor.matmul(
                        ps1[:, :], bd_t[:, :], tt_v[:, 8 * ch : 8 * ch + 8, :, :]
                    )
                    ut = midpool.tile([128, 512], f32, tag="ut")
                    nc.vector.transpose(ut[:, :], ps1[:, :])
                    ps2 = pp2.tile([128, 512], f32, tag="ps2")
                    nc.tensor.matmul(ps2[:, :], bd_t[:, :], ut[:, :])
                    # signed copy for the store (ACT)
                    nc.scalar.copy(sy[:, 512 * ch : 512 * ch + 512], ps2[:, :])
                nc.sync.dma_start(y[m], sy[:, :])

                # grade path, batched per megatile:
                # abs (ACT) -> ln1p (ACT) -> *W (gpsimd) -> PE row-sums
                ab = midpool.tile([128, F], f32, tag="ab")
                nc.scalar.activation(
                    ab[:, :], sy[:, :], mybir.ActivationFunctionType.Abs
                )
                la = midpool.tile([128, F], f32, tag="la")
                nc.scalar.activation(
                    la[:, :], ab[:, :], mybir.ActivationFunctionType.Ln, bias=1.0
                )
                lw = midpool.tile([128, F], f32, tag="lw")
                nc.gpsimd.tensor_tensor(
                    lw[:, :], la[:, :], wblk_t[:, :], mybir.AluOpType.mult
                )
                for ch in range(4):
                    # row-sum over i' into the megatile's grade rows
                    nc.tensor.matmul(
                        gbank[32 * ch : 32 * ch + 32, :],
                        ind_t[:, 32 * mp : 32 * mp + 32],
                        lw[:, 512 * ch : 512 * ch + 512],
                        start=(mp == 0),
                        stop=(mp == 7 or m == n_mega - 1),
                        tile_position=(0, 32 * ch),
                        skip_group_check=True,
                    )

            # final j' reduction of the grade banks
            gr_s = grpool.tile([128, 16 * n_banks], f32)
            for b in range(n_banks):
                gcop = grpool.tile([128, 512], f32, tag="gcop")
                nc.scalar.copy(gcop[:, :], gb[b][:, :])
                nc.vector.tensor_reduce(
                    gr_s[:, 16 * b : 16 * b + 16],
                    gcop.rearrange("p (g j) -> p g j", g=16, j=32),
                    axis=mybir.AxisListType.X,
                    op=mybir.AluOpType.add,
                )
            nc.sync.dma_start(grades[:, :], gr_s[:, :])
    nc.finalize()
    return nc



def build_nc_v3(n_mega=N_MEGA):
    """v3: 16-way tile_position-packed fp32 matmuls (3.5x PE throughput).

    Stage 1 (2 rounds of 8 tiles): tile (r,s) computes, for the 16
    patches {64r+16s+t}, U^T... actually U = D @ P^T:
      bank_r[32s+i, 32t+h] = U_{64r+16s+t}[i, h]
    Stage 2 (16 tiles): tile at position (32ch, 32r) reads ut_r rows
    32ch..32ch+31 and writes
      psum2[32r+i', 512ch+32g+j'] = Y_{64r+16ch+g}[i', j']
    Grade: per-megatile ind-matmuls (M=4) -> gbm -> DVE j'-reduce.
    Store: block layout (host de-transposes).
    """
    _ensure_path()
    import concourse.bacc as bacc
    import concourse.mybir as mybir
    from concourse.tile import TileContext

    f32 = mybir.dt.float32
    nc = bacc.Bacc("TRN2", target_bir_lowering=False)

    x = nc.dram_tensor("x", [n_mega, 128, F], f32, kind="ExternalInput")
    dt_in = nc.dram_tensor("dt", [128, 34], f32, kind="ExternalInput")
    wblk = nc.dram_tensor("wblk", [128, 2048], f32, kind="ExternalInput")
    ind = nc.dram_tensor("ind", [128, 32], f32, kind="ExternalInput")
    y = nc.dram_tensor("y", [n_mega, 128, F], f32, kind="ExternalOutput")
    grades = nc.dram_tensor("grades", [128, 16 * n_mega], f32, kind="ExternalOutput")

    with TileContext(nc) as tc:
        with (
            tc.tile_pool(name="const", bufs=1) as cpool,
            tc.tile_pool(name="io", bufs=3) as iopool,
            tc.tile_pool(name="mid", bufs=3) as midpool,
            tc.tile_pool(name="gr", bufs=1) as grpool,
            tc.tile_pool(name="pp1", bufs=1, space="PSUM") as pp1,
            tc.tile_pool(name="pp2", bufs=1, space="PSUM") as pp2,
            tc.tile_pool(name="ppg", bufs=2, space="PSUM") as ppg,
        ):
            dt_t = cpool.tile([128, 34], f32)
            nc.sync.dma_start(dt_t[:, :], dt_in[:, :])
            wblk_t = cpool.tile([128, 2048], f32)
            nc.sync.dma_start(wblk_t[:, :], wblk[:, :])
            ind_t = cpool.tile([128, 32], f32)
            nc.sync.dma_start(ind_t[:, :], ind[:, :])
            gr_s = grpool.tile([128, 16 * n_mega], f32)

            pending = []

            def emit_grade(gm, glw):
                gbm = ppg.tile([128, 512], f32, tag="gbm", name=f"gbm{gm}")
                for ch in range(4):
                    nc.tensor.matmul(
                        gbm[32 * ch : 32 * ch + 32, :],
                        ind_t[:, :],
                        glw[:, 512 * ch : 512 * ch + 512],
                        tile_position=(0, 32 * ch),
                        skip_group_check=True,
                    )
                nc.vector.tensor_reduce(
                    gr_s[:, 16 * gm : 16 * gm + 16],
                    gbm.rearrange("p (g j) -> p g j", g=16, j=32),
                    axis=mybir.AxisListType.X,
                    op=mybir.AluOpType.add,
                )

            for m in range(n_mega):
                tile_c = iopool.tile([128, F], f32, tag="tile_c")
                nc.sync.dma_start(tile_c[:, :], x[m])

                tt = midpool.tile([128, F], f32, tag="tt")
                nc.vector.transpose(tt[:, :], tile_c[:, :])
                # tt[32a+w, 32(32d+h)+c] = patch_{64a+2c+d}[h, w]
                # tile (r,s) rhs col for (t=2u+d, h): 1024d + 32h + 8s + u
                tt_v = tt.rearrange("p (d h c) -> p c d h", d=2, h=32, c=32)

                uts = []
                for r in range(4):
                    bank = pp1.tile(
                        [128, 512], f32, tag=f"b{r % 2}", name=f"bank{r}_{m}"
                    )
                    for s in range(4):
                        nc.tensor.matmul(
                            bank[32 * s : 32 * s + 32, :],
                            dt_t[32 * r : 32 * r + 32, :32],
                            tt_v[32 * r : 32 * r + 32, 8 * s : 8 * s + 8, :, :],
                            tile_position=(32 * r, 32 * s),
                            skip_group_check=True,
                        )
                    ut = midpool.tile([128, 512], f32, tag=f"ut{r}", name=f"ut{r}_{m}")
                    nc.vector.transpose(ut[:, :], bank[:, :])
                    uts.append(ut)

                ps2 = pp2.tile([128, F], f32, tag="ps2")
                for ch in range(4):
                    for r in range(4):
                        nc.tensor.matmul(
                            ps2[32 * r : 32 * r + 32, 512 * ch : 512 * ch + 512],
                            dt_t[32 * ch : 32 * ch + 32, :32],
                            uts[r][32 * ch : 32 * ch + 32, :],
                            tile_position=(32 * ch, 32 * r),
                            skip_group_check=True,
                        )

                sy = iopool.tile([128, F], f32, tag="sy")
                nc.scalar.copy(sy[:, :], ps2[:, :])
                nc.sync.dma_start(y[m], sy[:, :])

                ab = midpool.tile([128, F], f32, tag="ab")
                nc.scalar.activation(
                    ab[:, :], sy[:, :], mybir.ActivationFunctionType.Abs
                )
                la = midpool.tile([128, F], f32, tag="la")
                nc.scalar.activation(
                    la[:, :], ab[:, :], mybir.ActivationFunctionType.Ln, bias=1.0
                )
                lw = midpool.tile([128, F], f32, tag="lw", name=f"lw{m}")
                nc.gpsimd.tensor_tensor(
                    lw[:, :], la[:, :], wblk_t[:, :], mybir.AluOpType.mult
                )
                pending.append((m, lw))

                # emit the PREVIOUS megatile's grade matmuls here so the
                # in-order PE never waits on the abs->ln->mult chain
                if len(pending) > 1:
                    emit_grade(*pending.pop(0))

            while pending:
                emit_grade(*pending.pop(0))
            nc.sync.dma_start(grades[:, :], gr_s[:, :])
    nc.finalize()
    return nc



def build_nc_v4(n_mega=N_MEGA):
    """v4: software-pipelined v3.

    PE emission order per m: s1A(m) [12 tiles, r=0..2] -> s2(m-1) [16] ->
    s1B(m) [4 tiles, r=3] -> ind(m-2) [4], so the PE stream is continuous
    (keeps HAM un-throttled at 2.4 GHz) and never waits on the DVE
    transposes or the grade elementwise chain.
    PSUM: 3 stage-1 banks (r3 reuses b0) + 4 stage-2 + 1 grade = 8.
    """
    _ensure_path()
    import concourse.bacc as bacc
    import concourse.mybir as mybir
    from concourse.tile import TileContext

    f32 = mybir.dt.float32
    bf16 = mybir.dt.bfloat16
    nc = bacc.Bacc("TRN2", target_bir_lowering=False)

    x = nc.dram_tensor("x", [n_mega, 128, F], f32, kind="ExternalInput")
    dt_in = nc.dram_tensor("dt", [128, 34], f32, kind="ExternalInput")
    wblk = nc.dram_tensor("wblk", [128, 2048], f32, kind="ExternalInput")
    ind = nc.dram_tensor("ind", [128, 32], bf16, kind="ExternalInput")
    y = nc.dram_tensor("y", [n_mega, 128, F], f32, kind="ExternalOutput")
    grades = nc.dram_tensor("grades", [128, 16 * n_mega], f32, kind="ExternalOutput")

    with TileContext(nc) as tc:
        with (
            tc.tile_pool(name="const", bufs=1) as cpool,
            tc.tile_pool(name="io", bufs=3) as iopool,
            tc.tile_pool(name="mid", bufs=3) as midpool,
            tc.tile_pool(name="gr", bufs=1) as grpool,
            tc.tile_pool(name="pp1", bufs=1, space="PSUM") as pp1,
            tc.tile_pool(name="pp2", bufs=1, space="PSUM") as pp2,
            tc.tile_pool(name="ppg", bufs=1, space="PSUM") as ppg,
        ):
            dt_t = cpool.tile([128, 34], f32)
            wblk_t = cpool.tile([128, 2048], f32)
            ind_t = cpool.tile([128, 32], bf16)
            gr_s = grpool.tile([128, 16 * n_mega], f32)
            consts_loaded = []

            def emit_const_loads():
                nc.sync.dma_start(dt_t[:, :], dt_in[:, :])
                nc.sync.dma_start(ind_t[:, :], ind[:, :])
                nc.sync.dma_start(wblk_t[:, :], wblk[:, :])
                consts_loaded.append(True)

            BTAG = ["b0", "b1", "b2", "gbm"]

            def emit_load_t1(m, nchunk):
                tt = midpool.tile([128, F], f32, tag="tt", name=f"tt{m}")
                cw = F // nchunk
                for c in range(nchunk):
                    sl = slice(cw * c, cw * c + cw)
                    if nchunk > 1:
                        tc_c = iopool.tile(
                            [128, cw], f32, tag=f"tcc{c}", name=f"tcc{m}_{c}"
                        )
                        nc.sync.dma_start(tc_c[:, :], x[m, :, sl])
                        if not consts_loaded:
                            emit_const_loads()
                        nc.vector.transpose(tt[:, sl], tc_c[:, :])
                    else:
                        tile_c = iopool.tile(
                            [128, F], f32, tag="tile_c", name=f"tc{m}"
                        )
                        nc.sync.dma_start(tile_c[:, :], x[m])
                        if not consts_loaded:
                            emit_const_loads()
                        nc.vector.transpose(tt[:, :], tile_c[:, :])
                return tt.rearrange("p (d h c) -> p c d h", d=2, h=32, c=32)

            def emit_s1(m, tt_v, rs):
                outs = []
                for r in rs:
                    pool = ppg if r == 3 else pp1
                    bank = pool.tile(
                        [128, 512], f32, tag=BTAG[r], name=f"bank{r}_{m}"
                    )
                    for s in range(4):
                        nc.tensor.matmul(
                            bank[32 * s : 32 * s + 32, :],
                            dt_t[32 * r : 32 * r + 32, :32],
                            tt_v[32 * r : 32 * r + 32, 8 * s : 8 * s + 8, :, :],
                            tile_position=(32 * r, 32 * s),
                            skip_group_check=True,
                        )
                    ut = midpool.tile([128, 512], f32, tag=f"ut{r}", name=f"ut{r}_{m}")
                    nc.vector.transpose(ut[:, :], bank[:, :])
                    outs.append(ut)
                return outs

            def emit_s2(m, uts):
                ps2 = pp2.tile([128, F], f32, tag="ps2", name=f"ps2_{m}")
                for ch in range(4):
                    for r in range(4):
                        nc.tensor.matmul(
                            ps2[32 * r : 32 * r + 32, 512 * ch : 512 * ch + 512],
                            dt_t[32 * ch : 32 * ch + 32, :32],
                            uts[r][32 * ch : 32 * ch + 32, :],
                            tile_position=(32 * ch, 32 * r),
                            skip_group_check=True,
                        )
                return ps2

            def emit_tail(m, ps2, nchunk):
                sy = iopool.tile([128, F], f32, tag="sy", name=f"sy{m}")
                ab = midpool.tile([128, F], f32, tag="ab", name=f"ab{m}")
                la = midpool.tile([128, F], bf16, tag="la", name=f"la{m}")
                lw = midpool.tile([128, F], bf16, tag="lw", name=f"lw{m}", bufs=4)
                cw = F // nchunk
                for c in range(nchunk):
                    sl = slice(cw * c, cw * c + cw)
                    nc.scalar.copy(sy[:, sl], ps2[:, sl])
                    nc.sync.dma_start(y[m, :, sl], sy[:, sl])
                    nc.scalar.activation(
                        ab[:, sl],
                        sy[:, sl],
                        mybir.ActivationFunctionType.Abs,
                        bias=dt_t[:, 33:34],
                    )
                    nc.scalar.activation(
                        la[:, sl],
                        ab[:, sl],
                        mybir.ActivationFunctionType.Ln,
                        bias=dt_t[:, 32:33],
                    )
                    nc.gpsimd.tensor_tensor(
                        lw[:, sl], la[:, sl], wblk_t[:, sl], mybir.AluOpType.mult
                    )
                return lw

            def emit_grade(gm, glw):
                gbm = ppg.tile([128, 512], f32, tag="gbm", name=f"gbm{gm}")
                for ch in range(4):
                    nc.tensor.matmul(
                        gbm[32 * ch : 32 * ch + 32, :],
                        ind_t[:, :],
                        glw[:, 512 * ch : 512 * ch + 512],
                        tile_position=(0, 32 * ch),
                        skip_group_check=True,
                    )
                nc.vector.tensor_reduce(
                    gr_s[:, 16 * gm : 16 * gm + 16],
                    gbm.rearrange("p (g j) -> p g j", g=16, j=32),
                    axis=mybir.AxisListType.X,
                    op=mybir.AluOpType.add,
                )

            uts_prev = None
            lw_pend = []
            for m in range(n_mega):
                tt_v = emit_load_t1(m, 4 if m == 0 else 1)
                uts = emit_s1(m, tt_v, (0, 1, 2, 3))
                if uts_prev is not None:
                    ps2 = emit_s2(m - 1, uts_prev)
                    lw_pend.append((m - 1, emit_tail(m - 1, ps2, 1)))
                if len(lw_pend) > 2:
                    emit_grade(*lw_pend.pop(0))
                uts_prev = uts

            ps2 = emit_s2(n_mega - 1, uts_prev)
            lw_pend.append((n_mega - 1, emit_tail(n_mega - 1, ps2, 4)))
            while lw_pend:
                emit_grade(*lw_pend.pop(0))
            nc.sync.dma_start(grades[:, :], gr_s[:, :])
    nc.finalize()
    return nc


def make_consts_v3(dct_matrix, bandpass_filters):
    D = np.asarray(dct_matrix, np.float32)
    bf = np.asarray(bandpass_filters, np.float32)
    dt = np.tile(D.T, (4, 1)).astype(np.float32)           # [128, 32]
    dt = np.concatenate(
        [dt, np.ones((128, 1), np.float32), np.zeros((128, 1), np.float32)], axis=1
    )                                                      # [128, 34]
    kw = (2.0 ** np.arange(bf.shape[0])).astype(np.float32)
    W = np.einsum("k,khw->hw", kw, bf).astype(np.float32)
    wblk = np.tile(W, (4, 64)).astype(np.float32)          # [128, 2048]
    import ml_dtypes

    ind = np.zeros((128, 32), np.float32)
    for k in range(4):
        ind[32 * k : 32 * k + 32, k] = 1.0
    return dt, wblk, ind.astype(ml_dtypes.bfloat16)


def unscramble_grades_v3(gr, n_mega=N_MEGA):
    """[128, 16*n_mega] -> per-patch sums in t order.

    gr[32ch+k, 16m+g] = grade of patch q = 256m + 64k + 16ch + g
    (rows 32ch+4.. are garbage)."""
    a = gr.reshape(4, 32, n_mega, 16)[:, :4]   # (ch, k, m, g)
    a = a.transpose(2, 1, 0, 3).reshape(-1)    # (m, k, ch, g) -> t
    return a


def make_consts_v2(dct_matrix, bandpass_filters):
    D = np.asarray(dct_matrix, np.float32)
    bf = np.asarray(bandpass_filters, np.float32)
    BD = np.zeros((128, 128), np.float32)
    for k in range(4):
        BD[32 * k : 32 * k + 32, 32 * k : 32 * k + 32] = D.T
    kw = (2.0 ** np.arange(bf.shape[0])).astype(np.float32)
    W = np.einsum("k,khw->hw", kw, bf).astype(np.float32)
    wblk = np.tile(W, (4, 64)).astype(np.float32)          # [128, 2048]
    ind = np.zeros((128, 8, 32), np.float32)
    for k in range(4):
        for mp in range(8):
            ind[32 * k : 32 * k + 32, mp, 4 * mp + k] = 1.0
    return BD, wblk, np.ascontiguousarray(ind.reshape(128, 256))


def unscramble_y_v2(y_dev, n_mega=N_MEGA):
    """[n_mega,128,2048] block layout -> [npatch,32,32] in t order."""
    a = y_dev.reshape(n_mega, 4, 32, 4, 16, 32)  # (m, k, i, ch, g, j)
    a = a.transpose(0, 1, 3, 4, 2, 5)            # (m, k, ch, g, i, j)
    return np.ascontiguousarray(a).reshape(n_mega * 256, 32, 32)


def unscramble_grades_v2(gr, n_mega=N_MEGA):
    """[128, 16*n_banks] -> per-patch sums [n_mega*256] in t order."""
    n_banks = (n_mega + 7) // 8
    a = gr.reshape(4, 8, 4, n_banks, 16)         # (ch, mp, k, b, g)
    a = a.transpose(3, 1, 2, 0, 4).reshape(-1)   # t = 256*(8b+mp)+64k+16ch+g
    return a[: n_mega * 256]


def make_consts(dct_matrix, bandpass_filters):
    D = np.asarray(dct_matrix, np.float32)
    bf = np.asarray(bandpass_filters, np.float32)
    BD = np.zeros((128, 128), np.float32)
    for k in range(4):
        BD[32 * k : 32 * k + 32, 32 * k : 32 * k + 32] = D.T
    kw = (2.0 ** np.arange(bf.shape[0])).astype(np.float32)
    W = np.einsum("k,khw->hw", kw, bf).astype(np.float32)
    wrep = np.tile(W.reshape(1, 1024), (128, 1)).astype(np.float32)
    return BD, wrep


_NC_CACHE = {}


def _get_nc(n_mega=N_MEGA):
    if n_mega not in _NC_CACHE:
        _NC_CACHE[n_mega] = build_nc_v4(n_mega)
    return _NC_CACHE[n_mega]


def kernel(x, dct_matrix, bandpass_filters):
    _ensure_path()
    from concourse.bass_utils import run_bass_kernel_spmd

    x = np.asarray(x, np.float32)
    DT, wblk, ind = make_consts_v3(dct_matrix, bandpass_filters)
    nc = _get_nc()

    in_maps = []
    for c in range(N_CORES):
        shard = np.ascontiguousarray(
            x[c * B_SHARD : (c + 1) * B_SHARD]
        ).reshape(N_MEGA, 128, F)
        in_maps.append({"x": shard, "dt": DT, "wblk": wblk, "ind": ind})

    res = run_bass_kernel_spmd(nc, in_maps, core_ids=list(range(N_CORES)))

    dct_parts, grade_parts = [], []
    for c in range(N_CORES):
        yc = unscramble_y_v2(res.results[c]["y"]).reshape(
            B_SHARD, NP, C, PATCH, PATCH
        )
        dct_parts.append(yc)
        ps = unscramble_grades_v3(res.results[c]["grades"])  # t-order
        grade_parts.append(ps.reshape(B_SHARD, NP, C).sum(-1))
    dct_coeffs = np.concatenate(dct_parts, axis=0).astype(np.float32)
    patch_grades = np.concatenate(grade_parts, axis=0).astype(np.float32)
    return dct_coeffs, patch_grades
